# revision 20
# baseline (speedup 1.0000x reference)
"""Trainium2 Bass kernel for the attention-pooling layer (fp16 pipeline).

Computation (per sample b):
    q = input2 @ fc_w.T + fc_b                      # [B, C1]
    scores[b, p] = <input1[b, :, p], q[b]>          # [B, HW]
    attn = softmax(scores, axis=1)
    out[b, c] = sum_p input1[b, c, p] * attn[b, p]  # [B, C1]

Sharding: data-parallel over batch across 8 NeuronCores (8 samples each),
fc_w replicated (no collectives).

Precision: the 2e-2 rel-err budget admits fp16 inputs with fp32
accumulation end-to-end (measured 3.7e-3 against the fp32 reference).
fp16 halves HBM traffic for x (12.9MB/core) + fc_w (4.2MB/core), runs the
PE at 1 cycle/row (fp32 matmul costs 4), and doubles DVE throughput
(2x_1P mode) for the pooling multiplies.

Per-core phases (one TileContext; Tile inserts all semaphores):
  1. q: in2T-stationary matmuls (k-outer so accumulation chases the fc_w
     DMA chunks), bias folded in as a K=1 matmul, TensorE transpose to
     qT[c, b], cast fp16.
  2. scores (per sample): M=1 TensorE matmuls accumulating over the 8
     C1-chunks into one PSUM bank per 392-pixel half.
  3. softmax (per sample): negated reduce_max (DVE), one Exp with
     accum_out (ACT) -- a single table set for the whole kernel -- DVE
     reciprocal, GpSimd broadcasts of the attn row (fp16) and 1/sum (f32).
  4. pooling (per sample, per C1-chunk), balanced across three engines
     (fp16 tensor_tensor runs in 2x_1P DVE mode; scalar_tensor_tensor is
     1x-only, so most chunks go mult-on-DVE + accumulate-on-ACT):
       - 5 chunks: DVE tensor_tensor mult, ACT Copy with scale=1/sum and
         fused free-dim accum_out
       - 1 chunk: DVE scalar_tensor_tensor (self-contained accum)
       - 2 chunks: GpSimd tensor_tensor mult + the same ACT accumulate
     (all DVE ops here are single-port mode, so GpSimd never contends).

All 8 samples' x tiles stay resident in SBUF (100KB/partition), so the x
DMA stream (sync HWDGE ring) never stalls on buffer reuse; fc_w rides the
same ring first, small loads + per-sample output stores ride the ACT ring.
"""

import numpy as np

import concourse.bacc as bacc
import concourse.mybir as mybir
import concourse.tile as tile
from concourse import masks
from concourse.bass_utils import run_bass_kernel_spmd

F32 = mybir.dt.float32
F16 = mybir.dt.float16

B, C1, C2, HW = 64, 1024, 2048, 784
NCORES = 8
BL = B // NCORES          # samples per core
P = 128                   # partitions
CO = C1 // P              # 8 c1 chunks
KC = C2 // P              # 16 c2 chunks
HH = HW // 2              # 392, half the pixels (fits one PSUM bank)
XH = 2                    # x DMA split: halves of the c1-chunks per sample
COH = CO // XH            # c1-chunks per x half-tile
NTT = 2                   # pooling chunks: DVE tensor_tensor + ACT accum
NST = 4                   # pooling chunks: DVE scalar_tensor_tensor
NGP = CO - NTT - NST      # pooling chunks: GpSimd mult + ACT accum
WTC = 4                   # fc_w DMA chunks

_CACHE = {}


def _build(repeat=1):
    nc = bacc.Bacc("TRN2", target_bir_lowering=False, debug=False)

    x = nc.dram_tensor("x", [BL, P, CO * HW], F16, kind="ExternalInput").ap()
    wt = nc.dram_tensor("wt", [C2, C1], F16, kind="ExternalInput").ap()
    in2t = nc.dram_tensor("in2t", [P, KC * BL], F16, kind="ExternalInput").ap()
    fcb = nc.dram_tensor("fcb", [1, C1], F16, kind="ExternalInput").ap()
    out = nc.dram_tensor("out", [P, BL * CO], F32, kind="ExternalOutput").ap()
    with tile.TileContext(nc) as tc:
        _emit(tc, nc, x, wt, in2t, fcb, out, repeat=repeat)

    nc.compile()
    return nc


def _emit(tc, nc, x, wt, in2t, fcb, out, repeat=1):
    import contextlib

    ctx = contextlib.ExitStack()
    with ctx:
        const = ctx.enter_context(tc.tile_pool(name="const", bufs=1))
        wtp = ctx.enter_context(tc.tile_pool(name="wtp", bufs=1))
        xp = ctx.enter_context(tc.tile_pool(name="xp", bufs=BL * XH))
        sm = ctx.enter_context(tc.tile_pool(name="sm", bufs=3))

        # ---- small loads (ACT HWDGE ring) --------------------------------
        in2t_sb = const.tile([P, KC * BL], F16, name="in2t_sb", tag="in2t_sb")
        nc.scalar.dma_start(out=in2t_sb[:], in_=in2t)
        fcb_sb = const.tile([1, C1], F16, name="fcb_sb", tag="fcb_sb")
        nc.scalar.dma_start(out=fcb_sb[:], in_=fcb)
        ones_sb = const.tile([1, BL], F16, name="ones_sb", tag="ones_sb")
        nc.vector.memset(ones_sb[:], 1.0)
        ident = const.tile([P, P], F32, name="ident", tag="ident")
        masks.make_identity(nc, ident[:])

        q_sb = const.tile([BL, C1], F32, name="q_sb", tag="q_sb")
        qt_all = const.tile([P, CO, BL], F16, name="qt_all", tag="qt_all")
        gall = const.tile([P, BL * CO], F32, name="gall", tag="gall")

        wtr = wt.rearrange("(k p) c -> p k c", p=P)
        xr = x.rearrange("b p (h c q) -> b p h c q", h=XH, c=COH)

        for rep in range(repeat):
            # ---- big loads (sync HWDGE ring): fc_w first, then x ---------
            wt_sb = wtp.tile([P, KC, C1], F16, name="wt_sb", tag="wt_sb")
            kcw = KC // WTC
            for wch in range(WTC):
                ks = slice(wch * kcw, (wch + 1) * kcw)
                nc.sync.dma_start(out=wt_sb[:, ks, :], in_=wtr[:, ks, :])

            x_sb = []
            for b in range(BL):
                halves = []
                for h in range(XH):
                    t = xp.tile([P, COH, HW], F16, name="x_sb", tag="x_sb")
                    nc.sync.dma_start(out=t[:], in_=xr[b, :, h])
                    halves.append(t)
                x_sb.append(halves)

            # ---- phase 1: q[b, c] then qT[c, b] --------------------------
            with tc.tile_pool(name=f"q_pp{rep}", bufs=2, space="PSUM") as q_pp:
                q_ps = [
                    q_pp.tile([BL, 512], F32, name="q_ps", tag="q_ps")
                    for _ in range(2)
                ]
                for k in range(KC):
                    for h in range(2):
                        nc.tensor.matmul(
                            q_ps[h][:],
                            in2t_sb[:, k * BL:(k + 1) * BL],
                            wt_sb[:, k, h * 512:(h + 1) * 512],
                            start=(k == 0),
                            stop=False,
                        )
                for h in range(2):
                    nc.tensor.matmul(
                        q_ps[h][:],
                        ones_sb[0:1, 0:BL],
                        fcb_sb[0:1, h * 512:(h + 1) * 512],
                        start=False,
                        stop=True,
                    )
                    nc.vector.tensor_copy(
                        q_sb[:, h * 512:(h + 1) * 512], q_ps[h][:]
                    )
            with tc.tile_pool(name=f"qt_pp{rep}", bufs=4, space="PSUM") as qt_pp:
                for j in range(CO):
                    qt_ps = qt_pp.tile([P, BL], F32, name="qt_ps", tag="qt_ps")
                    nc.tensor.transpose(
                        qt_ps[:], q_sb[:, j * P:(j + 1) * P], ident[0:BL, 0:BL]
                    )
                    nc.vector.tensor_copy(qt_all[:, j, :], qt_ps[:])

            # ---- phases 2-4, software-pipelined --------------------------
            # pool(b-1) is emitted after softmax(b): the in-order DVE then
            # fills its exp/broadcast wait for sample b with sample b-1's
            # pooling instead of idling
            with tc.tile_pool(name=f"s_pp{rep}", bufs=4, space="PSUM") as s_pp:
                atiles = {}
                for b in range(BL):
                    atiles[("s_ps", b)] = _emit_scores(
                        nc, s_pp, x_sb, qt_all, b
                    )
                    _emit_softmax(nc, sm, b, atiles)
                    if b >= 1:
                        _emit_pool(nc, sm, x_sb, gall, b - 1, atiles)
                _emit_pool(nc, sm, x_sb, gall, BL - 1, atiles)
            # one store for all samples, from SP: its DMA FIFO is empty by
            # now and a waiting dma_start must not stall a busy engine's
            # sequencer (it would block every later instruction there)
            nc.sync.dma_start(out=out, in_=gall[:])


def _emit_scores(nc, s_pp, x_sb, qt_all, b):
    # 16 M=1 matmuls, co-outer/half-inner so the stationary q column is
    # reused across the two pixel halves
    s_ps = s_pp.tile([1, 2, 512], F32, name="s_ps", tag="s_ps")
    for co in range(CO):
        xt = x_sb[b][co // COH]
        for h in range(2):
            nc.tensor.matmul(
                s_ps[0:1, h, 0:HH],
                qt_all[:, co, b:b + 1],
                xt[:, co % COH, h * HH:(h + 1) * HH],
                start=(co == 0),
                stop=(co == CO - 1),
            )
    return s_ps


def _emit_softmax(nc, sm, b, atiles):
    s_ps = atiles.pop(("s_ps", b))
    nm = sm.tile([1, 1], F32, name="nm", tag="nm")
    nc.vector.tensor_reduce(
        nm[:], s_ps[0:1, :, 0:HH], axis=mybir.AxisListType.XY,
        op=mybir.AluOpType.max, negate=True,
    )
    l = sm.tile([1, 1], F32, name="l", tag="l")
    ar = sm.tile([1, HW], F16, name="ar", tag="ar")
    nc.scalar.activation(
        ar.rearrange("p (h n) -> p h n", h=2),
        s_ps[0:1, :, 0:HH],
        mybir.ActivationFunctionType.Exp,
        bias=nm[:], accum_out=l[:],
    )
    # a_sb only depends on the Exp output -- broadcast it before r_bc so
    # the GpSimd queue doesn't stall pooling on the DVE reciprocal
    a_sb = sm.tile([P, HW], F16, name="a_sb", tag="a_sb")
    nc.gpsimd.partition_broadcast(a_sb[:], ar[:])
    r = sm.tile([1, 1], F32, name="r", tag="r")
    nc.vector.reciprocal(r[:], l[:])
    r_bc = sm.tile([P, 1], F32, name="r_bc", tag="r_bc")
    nc.gpsimd.partition_broadcast(r_bc[:], r[:])
    atiles[("a", b)] = a_sb
    atiles[("r", b)] = r_bc


def _emit_pool(nc, sm, x_sb, gall, b, atiles):
    # 4 chunks DVE mult (2x fp16) + ACT accumulate w/ 1/sum scale,
    # 2 chunks DVE stt (self-contained), 2 chunks GpSimd stt
    a_sb = atiles.pop(("a", b))
    r_bc = atiles.pop(("r", b))

    def chunk(co):
        return x_sb[b][co // COH][:, co % COH, :]

    wa = sm.tile([P, HW], F16, name="wa", tag="wa", bufs=1)
    for co in range(NTT):
        wm = sm.tile([P, HW], F16, name="wm", tag="wm", bufs=2)
        nc.vector.tensor_tensor(
            out=wm[:], in0=chunk(co), in1=a_sb[:], op=mybir.AluOpType.mult
        )
        nc.scalar.activation(
            wa[:], wm[:], mybir.ActivationFunctionType.Copy,
            scale=r_bc[:],
            accum_out=gall[:, b * CO + co:b * CO + co + 1],
        )
    waste = sm.tile([P, HW], F16, name="waste", tag="waste", bufs=1)
    for cs in range(NST):
        co = NTT + cs
        nc.vector.scalar_tensor_tensor(
            out=waste[:], in0=chunk(co), scalar=r_bc[:], in1=a_sb[:],
            op0=mybir.AluOpType.mult, op1=mybir.AluOpType.mult,
            accum_out=gall[:, b * CO + co:b * CO + co + 1],
        )
    for cg in range(NGP):
        co = NTT + NST + cg
        wg = sm.tile([P, HW], F16, name="wg", tag="wg", bufs=2)
        nc.gpsimd.tensor_tensor(
            out=wg[:], in0=chunk(co), in1=a_sb[:], op=mybir.AluOpType.mult
        )
        nc.scalar.activation(
            wa[:], wg[:], mybir.ActivationFunctionType.Copy,
            scale=r_bc[:],
            accum_out=gall[:, b * CO + co:b * CO + co + 1],
        )


def _get_nc():
    if "nc" not in _CACHE:
        _CACHE["nc"] = _build()
    return _CACHE["nc"]


def _in_maps(input1, input2, fc_w, fc_b):
    input1 = np.asarray(input1, dtype=np.float32)
    input2 = np.asarray(input2, dtype=np.float32)
    fc_w = np.asarray(fc_w, dtype=np.float32)
    fc_b = np.asarray(fc_b, dtype=np.float32)

    wt = np.ascontiguousarray(fc_w.T.astype(np.float16))      # [C2, C1]
    fcb = np.ascontiguousarray(fc_b.reshape(1, C1).astype(np.float16))
    maps = []
    for i in range(NCORES):
        sl = slice(i * BL, (i + 1) * BL)
        # x[b, co*128+ci, q] -> [b, ci, co*HW+q]
        x_sh = np.ascontiguousarray(
            input1[sl]
            .reshape(BL, CO, P, HW)
            .transpose(0, 2, 1, 3)
            .reshape(BL, P, CO * HW)
            .astype(np.float16)
        )
        # in2t[p, k*BL + b] = input2[i*BL + b, k*128 + p]
        i2t = np.ascontiguousarray(
            input2[sl].T.reshape(KC, P, BL)
            .transpose(1, 0, 2)
            .reshape(P, KC * BL)
            .astype(np.float16)
        )
        maps.append({"x": x_sh, "wt": wt, "in2t": i2t, "fcb": fcb})
    return maps


def _assemble(results):
    outs = []
    for i in range(NCORES):
        arr = np.asarray(results[i]["out"])                 # [128, BL*CO]
        # arr[ci, b*CO + co] = g[b, co*128 + ci]
        outs.append(
            arr.reshape(P, BL, CO).transpose(1, 2, 0).reshape(BL, C1)
        )
    return np.ascontiguousarray(
        np.concatenate(outs, axis=0).astype(np.float32)
    )


def run(input1, input2, fc_w, fc_b, trace=False, **trace_kwargs):
    nc = _get_nc()
    res = run_bass_kernel_spmd(
        nc,
        _in_maps(input1, input2, fc_w, fc_b),
        core_ids=list(range(NCORES)),
        trace=trace,
        **trace_kwargs,
    )
    return _assemble(res.results), res


def kernel(input1, input2, fc_w, fc_b):
    out, _ = run(input1, input2, fc_w, fc_b)
    return out


# revision 29
# speedup vs baseline: 3.7983x; 3.7983x over previous
"""Trainium2 Bass kernel for the attention-pooling layer (fp16 pipeline).

Computation (per sample b):
    q = input2 @ fc_w.T + fc_b                      # [B, C1]
    scores[b, p] = <input1[b, :, p], q[b]>          # [B, HW]
    attn = softmax(scores, axis=1)
    out[b, c] = sum_p input1[b, c, p] * attn[b, p]  # [B, C1]

Sharding: data-parallel over batch across 8 NeuronCores (8 samples each),
fc_w replicated (no collectives).

Precision: the 2e-2 rel-err budget admits fp16 inputs with fp32
accumulation end-to-end (measured 3.7e-3 against the fp32 reference).
fp16 halves HBM traffic for x (12.9MB/core) + fc_w (4.2MB/core), runs the
PE at 1 cycle/row (fp32 matmul costs 4), and doubles DVE throughput
(2x_1P mode) for the pooling multiplies.

Per-core phases (one TileContext; Tile inserts all semaphores):
  1. q: in2T-stationary matmuls (k-outer so accumulation chases the fc_w
     DMA chunks), bias folded in as a K=1 matmul, TensorE transpose to
     qT[c, b], cast fp16.
  2. scores (per sample): M=1 TensorE matmuls accumulating over the 8
     C1-chunks into one PSUM bank per 392-pixel half.
  3. softmax (per sample): negated reduce_max (DVE), one Exp with
     accum_out (ACT) -- a single table set for the whole kernel -- DVE
     reciprocal, GpSimd broadcasts of the attn row (fp16) and 1/sum (f32).
  4. pooling (per sample, per C1-chunk), balanced across three engines
     (fp16 tensor_tensor runs in 2x_1P DVE mode; scalar_tensor_tensor is
     1x-only, so most chunks go mult-on-DVE + accumulate-on-ACT):
       - 5 chunks: DVE tensor_tensor mult, ACT Copy with scale=1/sum and
         fused free-dim accum_out
       - 1 chunk: DVE scalar_tensor_tensor (self-contained accum)
       - 2 chunks: GpSimd tensor_tensor mult + the same ACT accumulate
     (all DVE ops here are single-port mode, so GpSimd never contends).

All 8 samples' x tiles stay resident in SBUF (100KB/partition), so the x
DMA stream (sync HWDGE ring) never stalls on buffer reuse; fc_w rides the
same ring first, small loads + per-sample output stores ride the ACT ring.
"""

import numpy as np

import concourse.bacc as bacc
import concourse.mybir as mybir
import concourse.tile as tile
from concourse import masks
from concourse.bass_utils import run_bass_kernel_spmd

F32 = mybir.dt.float32
F16 = mybir.dt.float16

B, C1, C2, HW = 64, 1024, 2048, 784
NCORES = 8
BL = B // NCORES          # samples per core
P = 128                   # partitions
CO = C1 // P              # 8 c1 chunks
KC = C2 // P              # 16 c2 chunks
HH = HW // 2              # 392, half the pixels (fits one PSUM bank)
XH = 2                    # x DMA split: halves of the c1-chunks per sample
COH = CO // XH            # c1-chunks per x half-tile
NTT = 6                   # pooling chunks: DVE tensor_tensor + ACT accum
NST = 2                   # pooling chunks: DVE scalar_tensor_tensor
NTR = 0                   # pooling chunks: DVE tensor_tensor_reduce
NGP = CO - NTT - NST - NTR  # pooling chunks: GpSimd mult + ACT accum
# HW A/B (33-rep slope, ns/rep): (6,2,0,0)=63k (6,2) beats (4,4)=67k,
# (5,3)=70k, (2,4,0,2)=178k, (0,8)=107k, (0,5,0,3)=178k.  GpSimd fp16
# tensor_tensor is ~9us/chunk on silicon -- never route pooling there;
# native tensor_tensor_reduce crashes at runtime -- don't use it.
TT2 = False               # pair the NTT multiplies into [P, 2, HW] ops
WTC = 4                   # fc_w DMA chunks

_CACHE = {}


def _build(repeat=1):
    nc = bacc.Bacc("TRN2", target_bir_lowering=False, debug=False)

    x = nc.dram_tensor("x", [BL, P, CO * HW], F16, kind="ExternalInput").ap()
    wt = nc.dram_tensor("wt", [C2, C1], F16, kind="ExternalInput").ap()
    in2t = nc.dram_tensor("in2t", [P, KC * BL], F16, kind="ExternalInput").ap()
    fcb = nc.dram_tensor("fcb", [1, C1], F16, kind="ExternalInput").ap()
    out = nc.dram_tensor("out", [P, BL * CO], F32, kind="ExternalOutput").ap()
    with tile.TileContext(nc) as tc:
        _emit(tc, nc, x, wt, in2t, fcb, out, repeat=repeat)

    nc.compile()
    return nc


def _emit(tc, nc, x, wt, in2t, fcb, out, repeat=1):
    import contextlib

    ctx = contextlib.ExitStack()
    with ctx:
        const = ctx.enter_context(tc.tile_pool(name="const", bufs=1))
        wtp = ctx.enter_context(tc.tile_pool(name="wtp", bufs=1))
        xp = ctx.enter_context(tc.tile_pool(name="xp", bufs=BL * XH))
        sm = ctx.enter_context(tc.tile_pool(name="sm", bufs=3))

        # ---- small loads (ACT HWDGE ring) --------------------------------
        in2t_sb = const.tile([P, KC * BL], F16, name="in2t_sb", tag="in2t_sb")
        nc.scalar.dma_start(out=in2t_sb[:], in_=in2t)
        fcb_sb = const.tile([1, C1], F16, name="fcb_sb", tag="fcb_sb")
        nc.scalar.dma_start(out=fcb_sb[:], in_=fcb)
        ones_sb = const.tile([1, BL], F16, name="ones_sb", tag="ones_sb")
        nc.vector.memset(ones_sb[:], 1.0)
        ident = const.tile([P, P], F32, name="ident", tag="ident")
        masks.make_identity(nc, ident[:])

        q_sb = const.tile([BL, C1], F32, name="q_sb", tag="q_sb")
        qt_all = const.tile([P, CO, BL], F16, name="qt_all", tag="qt_all")
        gall = const.tile([P, BL * CO], F32, name="gall", tag="gall")
        gu = const.tile([P, BL * CO], F32, name="gu", tag="gu") \
            if NTR else None

        wtr = wt.rearrange("(k p) c -> p k c", p=P)
        xr = x.rearrange("b p (h c q) -> b p h c q", h=XH, c=COH)

        for rep in range(repeat):
            # ---- big loads (sync HWDGE ring): fc_w first, then x ---------
            wt_sb = wtp.tile([P, KC, C1], F16, name="wt_sb", tag="wt_sb")
            kcw = KC // WTC
            for wch in range(WTC):
                ks = slice(wch * kcw, (wch + 1) * kcw)
                nc.sync.dma_start(out=wt_sb[:, ks, :], in_=wtr[:, ks, :])

            x_sb = []
            for b in range(BL):
                halves = []
                for h in range(XH):
                    t = xp.tile([P, COH, HW], F16, name="x_sb", tag="x_sb")
                    nc.sync.dma_start(out=t[:], in_=xr[b, :, h])
                    halves.append(t)
                x_sb.append(halves)

            # ---- phase 1: q[b, c] then qT[c, b] --------------------------
            with tc.tile_pool(name=f"q_pp{rep}", bufs=2, space="PSUM") as q_pp:
                q_ps = [
                    q_pp.tile([BL, 512], F32, name="q_ps", tag="q_ps")
                    for _ in range(2)
                ]
                for k in range(KC):
                    for h in range(2):
                        nc.tensor.matmul(
                            q_ps[h][:],
                            in2t_sb[:, k * BL:(k + 1) * BL],
                            wt_sb[:, k, h * 512:(h + 1) * 512],
                            start=(k == 0),
                            stop=False,
                        )
                for h in range(2):
                    nc.tensor.matmul(
                        q_ps[h][:],
                        ones_sb[0:1, 0:BL],
                        fcb_sb[0:1, h * 512:(h + 1) * 512],
                        start=False,
                        stop=True,
                    )
                    nc.vector.tensor_copy(
                        q_sb[:, h * 512:(h + 1) * 512], q_ps[h][:]
                    )
            with tc.tile_pool(name=f"qt_pp{rep}", bufs=4, space="PSUM") as qt_pp:
                for j in range(CO):
                    qt_ps = qt_pp.tile([P, BL], F32, name="qt_ps", tag="qt_ps")
                    nc.tensor.transpose(
                        qt_ps[:], q_sb[:, j * P:(j + 1) * P], ident[0:BL, 0:BL]
                    )
                    nc.vector.tensor_copy(qt_all[:, j, :], qt_ps[:])

            # ---- phases 2-4, software-pipelined --------------------------
            # pool(b-1) is emitted after softmax(b): the in-order DVE then
            # fills its exp/broadcast wait for sample b with sample b-1's
            # pooling instead of idling
            with tc.tile_pool(name=f"s_pp{rep}", bufs=4, space="PSUM") as s_pp:
                atiles = {}
                for b in range(BL):
                    atiles[("s_ps", b)] = _emit_scores(
                        nc, s_pp, x_sb, qt_all, b
                    )
                    _emit_softmax(nc, sm, b, atiles)
                    if b >= 1:
                        _emit_pool(nc, sm, x_sb, gall, gu, b - 1, atiles)
                _emit_pool(nc, sm, x_sb, gall, gu, BL - 1, atiles)
            # one store for all samples, from SP: its DMA FIFO is empty by
            # now and a waiting dma_start must not stall a busy engine's
            # sequencer (it would block every later instruction there)
            nc.sync.dma_start(out=out, in_=gall[:])


def _emit_scores(nc, s_pp, x_sb, qt_all, b):
    # 16 M=1 matmuls, co-outer/half-inner so the stationary q column is
    # reused across the two pixel halves
    s_ps = s_pp.tile([1, 2, 512], F32, name="s_ps", tag="s_ps")
    for co in range(CO):
        xt = x_sb[b][co // COH]
        for h in range(2):
            nc.tensor.matmul(
                s_ps[0:1, h, 0:HH],
                qt_all[:, co, b:b + 1],
                xt[:, co % COH, h * HH:(h + 1) * HH],
                start=(co == 0),
                stop=(co == CO - 1),
            )
    return s_ps


def _emit_softmax(nc, sm, b, atiles):
    s_ps = atiles.pop(("s_ps", b))
    nm = sm.tile([1, 1], F32, name="nm", tag="nm")
    nc.vector.tensor_reduce(
        nm[:], s_ps[0:1, :, 0:HH], axis=mybir.AxisListType.XY,
        op=mybir.AluOpType.max, negate=True,
    )
    l = sm.tile([1, 1], F32, name="l", tag="l")
    ar = sm.tile([1, HW], F16, name="ar", tag="ar")
    nc.scalar.activation(
        ar.rearrange("p (h n) -> p h n", h=2),
        s_ps[0:1, :, 0:HH],
        mybir.ActivationFunctionType.Exp,
        bias=nm[:], accum_out=l[:],
    )
    # a_sb only depends on the Exp output -- broadcast it before r_bc so
    # the GpSimd queue doesn't stall pooling on the DVE reciprocal
    a_sb = sm.tile([P, HW], F16, name="a_sb", tag="a_sb")
    nc.gpsimd.partition_broadcast(a_sb[:], ar[:])
    r = sm.tile([1, 1], F32, name="r", tag="r")
    nc.vector.reciprocal(r[:], l[:])
    r_bc = sm.tile([P, 1], F32, name="r_bc", tag="r_bc")
    nc.gpsimd.partition_broadcast(r_bc[:], r[:])
    atiles[("a", b)] = a_sb
    atiles[("r", b)] = r_bc


def _emit_pool(nc, sm, x_sb, gall, gu, b, atiles):
    # 4 chunks DVE mult (2x fp16) + ACT accumulate w/ 1/sum scale,
    # 2 chunks DVE stt (self-contained), 2 chunks GpSimd stt
    a_sb = atiles.pop(("a", b))
    r_bc = atiles.pop(("r", b))

    def chunk(co):
        return x_sb[b][co // COH][:, co % COH, :]

    wa = sm.tile([P, HW], F16, name="wa", tag="wa", bufs=1)
    co = 0
    if TT2:
        # pair the DVE multiplies: one tensor_tensor over [P, 2, HW] with a
        # stride-0 broadcast of the attn row -- halves DVE init+drain count
        for _ in range(NTT // 2):
            hx, cl = co // COH, co % COH
            wm2 = sm.tile([P, 2, HW], F16, name="wm2", tag="wm2", bufs=2)
            a_b = a_sb[:].unsqueeze(1).broadcast_to((P, 2, HW))
            nc.vector.tensor_tensor(
                out=wm2[:], in0=x_sb[b][hx][:, cl:cl + 2, :], in1=a_b,
                op=mybir.AluOpType.mult,
            )
            for j in range(2):
                nc.scalar.activation(
                    wa[:], wm2[:, j, :], mybir.ActivationFunctionType.Copy,
                    scale=r_bc[:],
                    accum_out=gall[:, b * CO + co + j:b * CO + co + j + 1],
                )
            co += 2
    for _ in range(NTT - co):
        wm = sm.tile([P, HW], F16, name="wm", tag="wm", bufs=2)
        nc.vector.tensor_tensor(
            out=wm[:], in0=chunk(co), in1=a_sb[:], op=mybir.AluOpType.mult
        )
        nc.scalar.activation(
            wa[:], wm[:], mybir.ActivationFunctionType.Copy,
            scale=r_bc[:],
            accum_out=gall[:, b * CO + co:b * CO + co + 1],
        )
        co += 1
    waste = sm.tile([P, HW], F16, name="waste", tag="waste", bufs=1)
    for _ in range(NST):
        nc.vector.scalar_tensor_tensor(
            out=waste[:], in0=chunk(co), scalar=r_bc[:], in1=a_sb[:],
            op0=mybir.AluOpType.mult, op1=mybir.AluOpType.mult,
            accum_out=gall[:, b * CO + co:b * CO + co + 1],
        )
        co += 1
    ttr0 = co
    for _ in range(NTR):
        nc.vector.tensor_tensor_reduce(
            out=waste[:], in0=chunk(co), in1=a_sb[:],
            scale=1.0, scalar=0.0,
            op0=mybir.AluOpType.mult, op1=mybir.AluOpType.add,
            accum_out=gu[:, b * CO + co:b * CO + co + 1],
        )
        co += 1
    if NTR:
        # normalize the ttr chunks (their accum can't carry 1/sum)
        sl = slice(b * CO + ttr0, b * CO + ttr0 + NTR)
        nc.vector.scalar_tensor_tensor(
            out=gall[:, sl], in0=gu[:, sl], scalar=r_bc[:], in1=gu[:, sl],
            op0=mybir.AluOpType.mult, op1=mybir.AluOpType.bypass,
        )
    for _ in range(NGP):
        wg = sm.tile([P, HW], F16, name="wg", tag="wg", bufs=2)
        nc.gpsimd.tensor_tensor(
            out=wg[:], in0=chunk(co), in1=a_sb[:], op=mybir.AluOpType.mult
        )
        nc.scalar.activation(
            wa[:], wg[:], mybir.ActivationFunctionType.Copy,
            scale=r_bc[:],
            accum_out=gall[:, b * CO + co:b * CO + co + 1],
        )
        co += 1


def _get_nc():
    if "nc" not in _CACHE:
        _CACHE["nc"] = _build()
    return _CACHE["nc"]


def _in_maps(input1, input2, fc_w, fc_b):
    input1 = np.asarray(input1, dtype=np.float32)
    input2 = np.asarray(input2, dtype=np.float32)
    fc_w = np.asarray(fc_w, dtype=np.float32)
    fc_b = np.asarray(fc_b, dtype=np.float32)

    wt = np.ascontiguousarray(fc_w.T.astype(np.float16))      # [C2, C1]
    fcb = np.ascontiguousarray(fc_b.reshape(1, C1).astype(np.float16))
    maps = []
    for i in range(NCORES):
        sl = slice(i * BL, (i + 1) * BL)
        # x[b, co*128+ci, q] -> [b, ci, co*HW+q]
        x_sh = np.ascontiguousarray(
            input1[sl]
            .reshape(BL, CO, P, HW)
            .transpose(0, 2, 1, 3)
            .reshape(BL, P, CO * HW)
            .astype(np.float16)
        )
        # in2t[p, k*BL + b] = input2[i*BL + b, k*128 + p]
        i2t = np.ascontiguousarray(
            input2[sl].T.reshape(KC, P, BL)
            .transpose(1, 0, 2)
            .reshape(P, KC * BL)
            .astype(np.float16)
        )
        maps.append({"x": x_sh, "wt": wt, "in2t": i2t, "fcb": fcb})
    return maps


def _assemble(results):
    outs = []
    for i in range(NCORES):
        arr = np.asarray(results[i]["out"])                 # [128, BL*CO]
        # arr[ci, b*CO + co] = g[b, co*128 + ci]
        outs.append(
            arr.reshape(P, BL, CO).transpose(1, 2, 0).reshape(BL, C1)
        )
    return np.ascontiguousarray(
        np.concatenate(outs, axis=0).astype(np.float32)
    )


def run(input1, input2, fc_w, fc_b, trace=False, **trace_kwargs):
    nc = _get_nc()
    res = run_bass_kernel_spmd(
        nc,
        _in_maps(input1, input2, fc_w, fc_b),
        core_ids=list(range(NCORES)),
        trace=trace,
        **trace_kwargs,
    )
    return _assemble(res.results), res


def kernel(input1, input2, fc_w, fc_b):
    out, _ = run(input1, input2, fc_w, fc_b)
    return out
